# revision 42
# baseline (speedup 1.0000x reference)
"""DPFP delta-rule attention kernel for 8 Trainium2 NeuronCores (v2).

Sharding: core c = 2*bi + half handles batch bi, rows [half*2048, (half+1)*2048).
Each core projects k/v/beta for its local 2048 rows, accumulates the partial
fast-weight W in PSUM across all 16 row blocks (fp8 DoubleRow matmuls over
block pairs), then all-reduces W (bf16, 256KB) with its pair core while the
q pipeline runs, hiding the slow collective.

q is projected directly TRANSPOSED (features on partitions) with +/-
augmented weight columns, so the dpfp relu-concat needs no cross-partition
moves and no PE transposes. The dpfp circular shift (cat[s-1]) is a
shift-permutation matmul on the PE whose PSUM output feeds the product
multiply directly (SBUF<->SBUF DMA and DRAM bounces both proved far too
slow for this). The q normalizer (a sum over the 128 dpfp features =
partitions) comes from indicator-matrix matmuls accumulating all 16 heads
into one [16, 512] PSUM tile, reciprocal'd on DVE and broadcast back
across partitions with DRAM-staged partition-broadcast DMAs. Attention is
two M=64 matmuls per head pair into one PSUM bank (tile_position offsets 0
and 64), scaled+normalized+cast to fp8 in one DVE op.

The residual (64*(x + bout), preadded on host) enters the Wout PSUM
accumulation group via an identity matmul, and LayerNorm statistics are
computed straight from PSUM (the WV^2 factor folds into the eps bias of
the rsqrt; the normalize applies on the scalar engine as
Identity(z*rstd - mu*rstd)). Output blocks 0-7 are emitted between the
two attention halves so the LN/gamma/beta tail overlaps attention.

Scales: x is fp8 at unit scale; Wk columns x8, Wq/Wv/Wbeta/Wout x64, the
identity x64, the indicator matrix 1/SCALE; mmov folds an extra 1/16 so
fp8 stays in range even when ksum is small. All rescaling folds into
existing ops (W = psum/256; LN eps' = 4096*eps).
"""

import numpy as np
import ml_dtypes

import concourse.bass as bass
import concourse.mybir as mybir
import concourse.tile as tile
import bass_rust as _br
from concourse.bass_utils import run_bass_kernel_spmd

BF16 = mybir.dt.bfloat16
F32 = mybir.dt.float32
FP8 = mybir.dt.float8e4
NPBF = ml_dtypes.bfloat16
NPF8 = ml_dtypes.float8_e4m3

P = 128
HEADS = 16
DHEAD = 64
S = 128            # dpfp feature dim = 2 * nu * DHEAD
DIM = 1024
KO = 8             # 128-row contraction blocks in DIM
CKVB = 2 * DIM + HEADS  # k,v columns + beta columns in wkvb
LN_EPS = 1e-5
SCALE = 1.0 / DHEAD**0.5

WK = 4.0           # k-column scale
WV = 64.0          # v/beta/q/wout scale
WSQ = WK * WK      # prodk scale (cat is WK*cat_true)
WQ2 = WV * WV      # prodq scale

N_LOC = 2048
NBK = N_LOC // P   # 16 k/v blocks
NSC = 4            # superchunks of 512 rows
SCW = 512

AluOp = mybir.AluOpType
Act = mybir.ActivationFunctionType
AX = mybir.AxisListType
DR = mybir.MatmulPerfMode.DoubleRow

# ---------------------------------------------------------------------------
# Workarounds: this walrus build accepts at most ONE sync-wait per
# instruction. Tile attaches several (the tail drain waits on the whole
# global clock). Split extra waits onto preceding same-engine instructions,
# which execute in order, so the semantics are identical.
# ---------------------------------------------------------------------------

_NOPPABLE = {
    mybir.EngineType.SP,
    mybir.EngineType.PE,
    mybir.EngineType.DVE,
    mybir.EngineType.Pool,
    mybir.EngineType.Activation,
}


def _patched_drain_and_barrier(self, tick_clock, wait_clock):
    from concourse.tile import ScopedClock

    nc = self.nc
    drain_inst = nc.sync.drain()
    wait_clock.add_sem_waits(
        drain_inst.ins, ScopedClock({None: tick_clock.global_clock})
    )
    waits = list(drain_inst.ins.sync_info.on_wait or [])
    if len(waits) > 1:
        drain_inst.ins.sync_info.on_wait = waits[:1]
        for w in waits[1:]:
            extra = nc.sync.drain()
            extra.ins.sync_info = _br.SyncInfo(on_wait=[w], on_update=[])

    nc.all_engine_barrier()
    assert self.sems is not None
    popped = nc._tile_sem_poison_stack.pop()
    assert popped is self._sem_poison
    nc.clear_and_free_semaphores(list(self.sems.allocated().values()))
    nc.all_engine_barrier()


def _install_patches():
    tile.TileContext._drain_and_barrier = _patched_drain_and_barrier


def _split_multi_waits(nc):
    """Post-pass: leave at most one sync wait per instruction by hoisting
    extra waits onto new NoOps inserted immediately before, on the same
    engine queue."""
    n_new = 0
    for f in nc.m.functions:
        for bb in f.blocks:
            insts = bb.instructions
            out = []
            for ins in insts:
                si = ins.sync_info
                waits = list(si.on_wait) if si and si.on_wait else []
                if len(waits) > 1:
                    assert ins.engine in _NOPPABLE, (
                        f"multi-wait on unsupported engine {ins.engine}: {ins}"
                    )
                    for w in waits[:-1]:
                        n_new += 1
                        nop = _br.InstNoOp(
                            name=f"I-wsplit-{n_new}",
                            ins=[],
                            outs=[],
                            engine=ins.engine,
                        )
                        nop.sync_info = _br.SyncInfo(on_wait=[w], on_update=[])
                        out.append(nop)
                    si.on_wait = waits[-1:]
                out.append(ins)
            if len(out) != len(insts):
                insts[:] = out
    return n_new


# ---------------------------------------------------------------------------
# Program builder
# ---------------------------------------------------------------------------


def build_nc(split_waits=True, collective=True, debug=False, w8=True):
    _install_patches()
    nc = bass.Bass()

    xt_d = nc.dram_tensor("xt", [P, KO, N_LOC], FP8, kind="ExternalInput")
    xloc_d = nc.dram_tensor("xloc", [N_LOC, DIM], BF16, kind="ExternalInput")
    wkvb_d = nc.dram_tensor("wkvb", [P, KO, CKVB], FP8, kind="ExternalInput")
    wqa_d = nc.dram_tensor("wqa", [P, KO, 2 * DIM], FP8, kind="ExternalInput")
    wout_d = nc.dram_tensor("wout", [P, KO, DIM], FP8, kind="ExternalInput")
    w0_d = nc.dram_tensor("w0", [S, HEADS, DHEAD], F32, kind="ExternalInput")
    ident_d = nc.dram_tensor("ident64", [P, P], BF16, kind="ExternalInput")
    pshift_d = nc.dram_tensor("pshift", [P, P], BF16, kind="ExternalInput")
    emat_d = nc.dram_tensor("emat", [P, HEADS, HEADS], BF16, kind="ExternalInput")
    bbeta_d = nc.dram_tensor("bbeta64", [1, HEADS], F32, kind="ExternalInput")
    gamma_d = nc.dram_tensor("gamma", [1, DIM], F32, kind="ExternalInput")
    betaln_d = nc.dram_tensor("betaln", [1, DIM], F32, kind="ExternalInput")
    out_d = nc.dram_tensor("out_loc", [N_LOC, DIM], F32, kind="ExternalOutput")
    if debug:
        dbg_w = nc.dram_tensor("dbg_w", [S, HEADS * DHEAD], F32, kind="ExternalOutput")
        dbg_att = nc.dram_tensor("dbg_att", [P, 8, N_LOC], F32, kind="ExternalOutput")

    with tile.TileContext(nc) as tc:
        with (
            tc.tile_pool(name="singles", bufs=1) as singles,
            tc.tile_pool(name="ccdram", bufs=1, space="DRAM") as ccd,
        ):
            xt = singles.tile([P, KO, N_LOC], FP8)
            wkvb = singles.tile([P, KO, CKVB], FP8)
            # first k-block needs xt cols 0:128 and wkvb k-columns: load in
            # waves so the first projection group starts ~10us sooner
            for ko in range(KO):
                nc.sync.dma_start(xt[:, ko, 0:512], xt_d[:, ko, 0:512])
            for ko in range(KO):
                nc.sync.dma_start(wkvb[:, ko, 0:512], wkvb_d[:, ko, 0:512])
            for ko in range(KO):
                nc.sync.dma_start(wkvb[:, ko, 512:DIM], wkvb_d[:, ko, 512:DIM])
            for ko in range(KO):
                nc.sync.dma_start(
                    wkvb[:, ko, DIM:CKVB], wkvb_d[:, ko, DIM:CKVB]
                )
            for w in range(1, 4):
                for ko in range(KO):
                    nc.sync.dma_start(
                        xt[:, ko, w * 512 : (w + 1) * 512],
                        xt_d[:, ko, w * 512 : (w + 1) * 512],
                    )
            wqa = singles.tile([P, KO, 2 * DIM], FP8)
            for ko in range(KO):
                nc.sync.dma_start(wqa[:, ko, :], wqa_d[:, ko, :])
            wout = singles.tile([P, KO, DIM], FP8)
            for ko in range(KO):
                nc.sync.dma_start(wout[:, ko, :], wout_d[:, ko, :])
            w0 = singles.tile([S, HEADS, DHEAD], F32)
            nc.sync.dma_start(w0[:], w0_d[:])
            ident = singles.tile([P, P], BF16)
            nc.sync.dma_start(ident[:], ident_d[:])
            pshift = singles.tile([P, P], BF16)
            nc.sync.dma_start(pshift[:], pshift_d[:])
            emat = singles.tile([P, HEADS, HEADS], BF16)
            nc.sync.dma_start(emat[:], emat_d[:])
            bbeta = singles.tile([P, HEADS], F32)
            nc.gpsimd.dma_start(bbeta[:], bbeta_d[0].partition_broadcast(P))
            gamma = singles.tile([P, DIM], BF16)
            nc.gpsimd.dma_start(gamma[:], gamma_d[0].partition_broadcast(P))
            betaln = singles.tile([P, DIM], F32)
            nc.gpsimd.dma_start(betaln[:], betaln_d[0].partition_broadcast(P))
            eps_sb = singles.tile([P, 1], F32)
            nc.vector.memset(eps_sb[:], LN_EPS * WV * WV)
            w_bf = singles.tile([S, HEADS, DHEAD], BF16)
            att_sb = singles.tile([P, 8, N_LOC], FP8)

            # ================= phase K: k/v/beta, build partial W ========
            # prodk/mmov are packed fp8 into block-pair tiles so the W
            # accumulation runs as DoubleRow matmuls over 256 rows.
            with (
                tc.tile_pool(name="kwork", bufs=2) as kwork,
                tc.tile_pool(name="kpair", bufs=3) as kpair,
                tc.tile_pool(name="ksmall", bufs=4) as ksmall,
                tc.tile_pool(name="pp_kv", bufs=1, space="PSUM") as pp_kv,
                tc.tile_pool(name="pp_b", bufs=1, space="PSUM") as pp_b,
                tc.tile_pool(name="pp_w", bufs=1, space="PSUM") as pp_w,
            ):
                pw = pp_w.tile([S, HEADS * DHEAD], F32, tag="pw")
                wpend = []
                for blk in range(NBK):
                    r0 = blk * P
                    t = blk % 2
                    if t == 0:
                        kdt = FP8 if w8 else BF16
                        prodk8 = kpair.tile([P, 2, HEADS, S], kdt, tag="prodk8")
                        mmov8 = kpair.tile([P, 2, HEADS, DHEAD], kdt, tag="mmov8")
                    cat = kwork.tile([P, HEADS, S], BF16, tag="cat")
                    prodk = prodk8[:, t, :, :]
                    mmov = mmov8[:, t, :, :]
                    # k chunks (cols 0..1024 of wkvb, scaled x8)
                    for c in range(2):
                        ps = pp_kv.tile([P, 512], F32, tag="psk", bufs=2)
                        for ko in range(0, KO, 2):
                            nc.tensor.matmul(
                                ps[:],
                                xt[:, ko : ko + 2, r0 : r0 + P],
                                wkvb[:, ko : ko + 2, c * 512 : (c + 1) * 512],
                                start=(ko == 0), stop=(ko == KO - 2),
                                perf_mode=DR,
                            )
                        ps3 = ps.rearrange("p (h d) -> p h d", h=8)
                        h0 = c * 8
                        nc.scalar.activation(
                            cat[:, h0 : h0 + 8, 0:DHEAD], ps3, Act.Relu
                        )
                        nc.scalar.activation(
                            cat[:, h0 : h0 + 8, DHEAD:S], ps3, Act.Relu, scale=-1.0
                        )
                    # beta (cols 2048..2064, scaled x64)
                    psb = pp_b.tile([P, HEADS], F32, tag="psb")
                    for ko in range(0, KO, 2):
                        nc.tensor.matmul(
                            psb[:],
                            xt[:, ko : ko + 2, r0 : r0 + P],
                            wkvb[:, ko : ko + 2, 2 * DIM : 2 * DIM + HEADS],
                            start=(ko == 0), stop=(ko == KO - 2),
                            perf_mode=DR,
                        )
                    # bsb = psb + bbeta64 first, freeing the beta psum
                    bsb = ksmall.tile([P, HEADS], F32, tag="bsb")
                    nc.vector.tensor_add(bsb[:], psb[:], bbeta[:])
                    # dpfp products: prod[s] = cat[s] * cat[s-1 mod S]
                    nc.vector.tensor_mul(
                        prodk[:, :, 1:S], cat[:, :, 1:S], cat[:, :, 0 : S - 1]
                    )
                    nc.vector.tensor_mul(
                        prodk[:, :, 0:1], cat[:, :, 0:1], cat[:, :, S - 1 : S]
                    )
                    sk = ksmall.tile([P, HEADS], F32, tag="sk")
                    nc.vector.reduce_sum(sk[:], prodk, axis=AX.X)
                    ck = ksmall.tile([P, HEADS], F32, tag="ck")
                    nc.vector.reciprocal(ck[:], sk[:])
                    ak = ksmall.tile([P, HEADS], F32, tag="ak")
                    nc.vector.tensor_mul(ak[:], bsb[:], ck[:])
                    # v chunks -> mmov = ps_v * ak / 16
                    for c in range(2):
                        ps = pp_kv.tile([P, 512], F32, tag="psv", bufs=3)
                        for ko in range(0, KO, 2):
                            nc.tensor.matmul(
                                ps[:],
                                xt[:, ko : ko + 2, r0 : r0 + P],
                                wkvb[:, ko : ko + 2,
                                     DIM + c * 512 : DIM + (c + 1) * 512],
                                start=(ko == 0), stop=(ko == KO - 2),
                                perf_mode=DR,
                            )
                        ps3 = ps.rearrange("p (h d) -> p h d", h=8)
                        h0 = c * 8
                        nc.vector.scalar_tensor_tensor(
                            mmov[:, h0 : h0 + 8, :],
                            ps3,
                            1.0 / 16.0,
                            ak[:, h0 : h0 + 8, None].to_broadcast([P, 8, DHEAD]),
                            op0=AluOp.mult, op1=AluOp.mult,
                        )
                    # W partial accumulation over block pairs, fp8 DoubleRow
                    # (psum groups interleave across pairs; cells are
                    # disjoint per head region). Emission is delayed one
                    # pair so the PE never waits on this pair's mmov chain.
                    if w8 and t == 1:
                        pair = blk // 2
                        wpend.append((pair, prodk8, mmov8))
                        for dpair, dprod, dmmov in (
                            wpend if pair == NBK // 2 - 1 else [wpend.pop(0)]
                        ):
                            for h in range(HEADS):
                                nc.tensor.matmul(
                                    pw[:, h * DHEAD : (h + 1) * DHEAD],
                                    dprod[:, :, h, :],
                                    dmmov[:, :, h, :],
                                    start=(dpair == 0),
                                    stop=(dpair == NBK // 2 - 1),
                                    perf_mode=DR,
                                    skip_group_check=True,
                                )
                        if pair == NBK // 2 - 1:
                            wpend.clear()
                    elif not w8:
                        for h in range(HEADS):
                            nc.tensor.matmul(
                                pw[:, h * DHEAD : (h + 1) * DHEAD],
                                prodk[:, h, :],
                                mmov[:, h, :],
                                start=(blk == 0), stop=(blk == NBK - 1),
                                skip_group_check=True,
                            )

                # ---- finalize W: bf16 all-reduce with pair core ----------
                wsum = singles.tile([S, HEADS * DHEAD], BF16)
                nc.vector.tensor_copy(wsum[:], pw[:])
            if collective:
                w_ib = ccd.tile([S, HEADS * DHEAD], BF16, tag="w_ib")
                w_ob = ccd.tile([S, HEADS * DHEAD], BF16, tag="w_ob")
                nc.gpsimd.dma_start(w_ib[:], wsum[:])
                nc.gpsimd.collective_compute(
                    "AllReduce",
                    AluOp.add,
                    replica_groups=[[0, 1], [2, 3], [4, 5], [6, 7]],
                    ins=[w_ib.opt()],
                    outs=[w_ob.opt()],
                )
                wred = singles.tile([S, HEADS * DHEAD], BF16)
                # on the Pool queue so the SP queue never waits on the CC
                nc.gpsimd.dma_start(wred[:], w_ob[:])
            else:
                wred = wsum

            # ================= phase Q: qT, dpfp, qsum ====================
            # (emitted after the collective; nothing here depends on W, so
            # it all overlaps the all-reduce. Queues: PE proj/qsum, scalar
            # relu, SP DMAs, DVE products/recips. The dpfp circular shift
            # goes through DRAM: SBUF->SBUF DMA is slow while the DRAM path
            # is fast, and the shifted re-read stays linear. Quarters are
            # software-pipelined: quarter q's product/qsum work is emitted
            # after quarter q+1's projection so the PE never waits on the
            # DRAM bounce.)
            with (
                tc.tile_pool(name="qcat", bufs=2) as qcat,
                tc.tile_pool(name="prodp", bufs=1) as prodp,
                tc.tile_pool(name="qsmall", bufs=4) as qsmall,
                tc.tile_pool(name="bcp", bufs=4) as bcp,
                tc.tile_pool(name="sdram", bufs=2, space="DRAM") as sdram,
                tc.tile_pool(name="xq", bufs=4) as xq_pool,
                tc.tile_pool(name="ow", bufs=4) as ow,
                tc.tile_pool(name="osmall", bufs=4) as osmall,
                tc.tile_pool(name="pp_q", bufs=2, space="PSUM") as pp_q,
                tc.tile_pool(name="pp_qs", bufs=1, space="PSUM") as pp_qs,
                tc.tile_pool(name="pp_a", bufs=2, space="PSUM") as pp_a,
            ):
                state = {}

                def stage_a(quarter):
                    half, sc = quarter // 2, quarter % 2
                    q0 = half * 1024 + sc * SCW
                    if sc == 0:
                        prodT_t = prodp.tile(
                            [S, HEADS, 2 * SCW], BF16, tag="prodT",
                            name=f"prodT{half}",
                        )
                        state[f"prodT{half}"] = prodT_t
                        stg_t = sdram.tile(
                            [HEADS, 2 * SCW], BF16, tag="stg",
                            name=f"stg{half}",
                        )
                        state[f"stg{half}"] = stg_t
                    prodT = state[f"prodT{half}"]
                    catT = qcat.tile([S, HEADS, SCW], BF16, tag="catT")
                    # per head: project, relu, then a shift-permutation
                    # matmul (catTs = roll(catT, 1) on partitions) one head
                    # behind, and the dpfp product straight off its PSUM
                    def shift_mul(h):
                        psh = pp_q.tile([S, SCW], F32, tag="psh")
                        nc.tensor.matmul(
                            psh[:], pshift[:], catT[:, h, :],
                            start=True, stop=True,
                            skip_group_check=True,
                        )
                        nc.vector.tensor_mul(
                            prodT[:, h, sc * SCW : (sc + 1) * SCW],
                            catT[:, h, :], psh[:],
                        )
                    for h in range(HEADS):
                        psq = pp_q.tile([S, SCW], F32, tag="psq", bufs=3)
                        for ko in range(0, KO, 2):
                            nc.tensor.matmul(
                                psq[:],
                                wqa[:, ko : ko + 2, h * P : (h + 1) * P],
                                xt[:, ko : ko + 2, q0 : q0 + SCW],
                                start=(ko == 0), stop=(ko == KO - 2),
                                perf_mode=DR,
                            )
                        nc.scalar.activation(catT[:, h, :], psq[:], Act.Relu)
                        if h > 0:
                            shift_mul(h - 1)
                    shift_mul(HEADS - 1)

                def stage_b(quarter):
                    half, sc = quarter // 2, quarter % 2
                    prodT = state[f"prodT{half}"]
                    # qsum via indicator matmuls: qs[h, n] = sum_s prodT
                    qs = pp_qs.tile([HEADS, SCW], F32, tag="qs")
                    for h in range(HEADS):
                        nc.tensor.matmul(
                            qs[:],
                            emat[:, h, :],
                            prodT[:, h, sc * SCW : (sc + 1) * SCW],
                            start=(h == 0), stop=(h == HEADS - 1),
                            skip_group_check=True,
                        )
                    rc = qsmall.tile([HEADS, SCW], BF16, tag="rc")
                    with nc.allow_low_precision(reason="bf16 qsum recip"):
                        nc.vector.reciprocal(rc[:], qs[:])
                    nc.sync.dma_start(
                        state[f"stg{half}"][:, sc * SCW : (sc + 1) * SCW], rc[:]
                    )

                def stage_att(half):
                    # attention: att^T[d, n] per head pair, one psum bank
                    prodT = state[f"prodT{half}"]
                    stg = state[f"stg{half}"]
                    for g in range(8):
                        # bc[p, n] = 1/qsum of head (2g + p//64), row n
                        bc = bcp.tile([P, 2 * SCW], BF16, tag="bc")
                        nc.sync.dma_start(
                            bc[0:64, :], stg[2 * g].partition_broadcast(64)
                        )
                        nc.sync.dma_start(
                            bc[64:P, :], stg[2 * g + 1].partition_broadcast(64)
                        )
                        for sc in range(2):
                            q0 = half * 1024 + sc * SCW
                            pa = pp_a.tile([P, SCW], F32, tag="pa")
                            for r in range(2):
                                h = 2 * g + r
                                nc.tensor.matmul(
                                    pa[r * DHEAD : (r + 1) * DHEAD, :],
                                    w_bf[:, h, :],
                                    prodT[:, h, sc * SCW : (sc + 1) * SCW],
                                    start=True, stop=True,
                                    skip_group_check=True,
                                )
                            nc.vector.tensor_mul(
                                att_sb[:, g, q0 : q0 + SCW],
                                pa[:],
                                bc[:, sc * SCW : (sc + 1) * SCW],
                            )


                def out_block(blk):
                    r0 = blk * P
                    xl = xq_pool.tile([P, DIM], BF16, tag="xl")
                    nc.scalar.dma_start(xl[:], xloc_d[r0 : r0 + P, :])
                    zps = []
                    for c in range(2):
                        psz = pp_q.tile(
                            [P, 512], F32, tag=("psq" if c == 0 else "psh"),
                            bufs=(3 if c == 0 else 2),
                        )
                        for g in range(0, 8, 2):
                            nc.tensor.matmul(
                                psz[:],
                                att_sb[:, g : g + 2, r0 : r0 + P],
                                wout[:, g : g + 2, c * 512 : (c + 1) * 512],
                                start=(g == 0), stop=False,
                                perf_mode=DR,
                                skip_group_check=True,
                            )
                        # residual: += 64 * (x + bout)  (ident is 64*I)
                        nc.tensor.matmul(
                            psz[:],
                            ident[:],
                            xl[:, c * 512 : (c + 1) * 512],
                            start=False, stop=True,
                            skip_group_check=True,
                        )
                        zps.append(psz)
                    # LayerNorm straight from PSUM (z = WV * z_true; the eps
                    # in eps_sb carries the WV^2 factor so rstd_raw works on
                    # raw psum stats)
                    st = osmall.tile([P, 2, 6], F32, tag="st")
                    nc.vector.bn_stats(st[:, 0, :], zps[0][:])
                    nc.vector.bn_stats(st[:, 1, :], zps[1][:])
                    mv = osmall.tile([P, 2], F32, tag="mv")
                    nc.vector.bn_aggr(mv[:], st[:])
                    rstd = osmall.tile([P, 1], F32, tag="rstd")
                    nc.scalar.activation(
                        rstd[:], mv[:, 1:2], Act.Sqrt, bias=eps_sb[:]
                    )
                    nc.vector.reciprocal(rstd[:], rstd[:])
                    nmr = osmall.tile([P, 1], F32, tag="nmr")
                    nc.vector.scalar_tensor_tensor(
                        nmr[:], mv[:, 0:1], -1.0, rstd[:],
                        op0=AluOp.mult, op1=AluOp.mult,
                    )
                    zn = ow.tile([P, DIM], BF16, tag="zn")
                    for c in range(2):
                        nc.scalar.activation(
                            zn[:, c * 512 : (c + 1) * 512],
                            zps[c][:],
                            Act.Identity,
                            scale=rstd[:],
                            bias=nmr[:],
                        )
                    zg = ow.tile([P, DIM], BF16, tag="zg")
                    nc.vector.scalar_tensor_tensor(
                        zg[:], zn[:], 1.0, gamma[:],
                        op0=AluOp.mult, op1=AluOp.mult,
                    )
                    zo = ow.tile([P, DIM], F32, tag="zo")
                    nc.gpsimd.tensor_add(zo[:], zg[:], betaln[:])
                    nc.sync.dma_start(out_d[r0 : r0 + P, :], zo[:])

                stage_a(0)
                stage_a(1)
                stage_b(0)
                stage_a(2)
                stage_b(1)
                # w_bf = wred / 256 + w0  (pw carries WK^2*16*W_true); on the
                # DVE queue only now, after the CC-independent DVE work
                nc.vector.scalar_tensor_tensor(
                    w_bf[:],
                    wred.rearrange("p (h d) -> p h d", h=HEADS),
                    1.0 / 256.0,
                    w0[:],
                    op0=AluOp.mult, op1=AluOp.add,
                )
                stage_att(0)
                stage_a(3)
                stage_b(2)
                stage_b(3)
                for blk in range(8):
                    out_block(blk)
                stage_att(1)
                for blk in range(8, NBK):
                    out_block(blk)

            if debug:
                da = singles.tile([P, 8, N_LOC], F32)
                nc.vector.tensor_copy(da[:], att_sb[:])
                nc.sync.dma_start(dbg_att[:], da[:])

    if split_waits:
        _split_multi_waits(nc)
    return nc


# ---------------------------------------------------------------------------
# Host side
# ---------------------------------------------------------------------------


def _prep_shared(Wqkv, Wbeta, bbeta, Wout, bout, gamma, beta_ln):
    wq = Wqkv[:, 0:DIM]
    wk = Wqkv[:, DIM : 2 * DIM]
    wv = Wqkv[:, 2 * DIM : 3 * DIM]
    wkvb = np.concatenate([wk * WK, wv * WV, Wbeta * WV], axis=1).astype(NPF8)
    wkvb = np.ascontiguousarray(wkvb.reshape(KO, P, CKVB).transpose(1, 0, 2))
    wqa = np.empty((DIM, 2 * DIM), dtype=np.float32)
    for h in range(HEADS):
        cols = wq[:, h * DHEAD : (h + 1) * DHEAD] * WV
        wqa[:, h * P : h * P + DHEAD] = cols
        wqa[:, h * P + DHEAD : (h + 1) * P] = -cols
    wqa = np.ascontiguousarray(
        wqa.astype(NPF8).reshape(KO, P, 2 * DIM).transpose(1, 0, 2)
    )
    wout = np.ascontiguousarray(
        (Wout * WV).astype(NPF8).reshape(KO, P, DIM).transpose(1, 0, 2)
    )
    ident = (np.eye(P, dtype=np.float32) * WV).astype(NPBF)
    pshift = np.roll(np.eye(P, dtype=np.float32), 1, axis=1).astype(NPBF)
    # att_ps and qs both carry the WQ2 = WV^2 prodq scale, which cancels in
    # att_ps * (1/qs); emat carries 1/SCALE so att_sb lands at true scale.
    emat = np.zeros((P, HEADS, HEADS), dtype=np.float32)
    for h in range(HEADS):
        emat[:, h, h] = 1.0 / SCALE
    return {
        "wkvb": wkvb,
        "wqa": wqa,
        "wout": wout,
        "ident64": np.ascontiguousarray(ident),
        "pshift": np.ascontiguousarray(pshift),
        "emat": np.ascontiguousarray(emat.astype(NPBF)),
        "bbeta64": np.ascontiguousarray(bbeta[None, :] * WV, dtype=np.float32),
        "gamma": np.ascontiguousarray(gamma[None, :], dtype=np.float32),
        "betaln": np.ascontiguousarray(beta_ln[None, :], dtype=np.float32),
        "_bout": np.asarray(bout, np.float32),
    }


def _prep_core(x, W0, bi, half, shared):
    loc = x[bi, half * N_LOC : (half + 1) * N_LOC]
    xt = np.ascontiguousarray(
        loc.T.astype(NPF8).reshape(KO, P, N_LOC).transpose(1, 0, 2)
    )
    m = {k: v for k, v in shared.items() if not k.startswith("_")}
    m["xt"] = xt
    m["xloc"] = np.ascontiguousarray(
        (loc + shared["_bout"][None, :]).astype(NPBF)
    )
    m["w0"] = np.ascontiguousarray(W0[bi].transpose(1, 0, 2), dtype=np.float32)
    return m


_NC = None


import os


def _get_nc():
    global _NC
    if _NC is None:
        _NC = build_nc(w8=os.environ.get('K_W8', '1') == '1')
    return _NC


def kernel(
    x, Wqkv, Wbeta, bbeta, Wout, bout, gamma, beta_ln, W0, _trace=False
):
    x = np.asarray(x, dtype=np.float32)
    b, n, _ = x.shape
    shared = _prep_shared(
        np.asarray(Wqkv, np.float32),
        np.asarray(Wbeta, np.float32),
        np.asarray(bbeta, np.float32),
        np.asarray(Wout, np.float32),
        np.asarray(bout, np.float32),
        np.asarray(gamma, np.float32),
        np.asarray(beta_ln, np.float32),
    )
    W0 = np.asarray(W0, np.float32)
    in_maps = []
    for c in range(8):
        bi, half = c // 2, c % 2
        in_maps.append(_prep_core(x, W0, bi, half, shared))

    nc = _get_nc()
    br = run_bass_kernel_spmd(nc, in_maps, core_ids=list(range(8)), trace=_trace)

    out = np.empty((b, n, DIM), dtype=np.float32)
    for c in range(8):
        bi, half = c // 2, c % 2
        out[bi, half * N_LOC : (half + 1) * N_LOC] = br.results[c]["out_loc"]
    if _trace:
        return out, br
    return out
